# revision 1
# baseline (speedup 1.0000x reference)
"""Multi-head attention Bass kernel for Trainium2, SPMD over 8 NeuronCores.

Problem: B=4, S=2048, D=1024, 16 heads x 64. Sharding: core = (batch b, head-group hg)
with b in 0..3, hg in 0..1 -> each core computes 8 heads of one batch.

The PE array is treated as a 4x4 grid of 32x32 sub-arrays; concurrent
matmuls with disjoint (row-group, col-group) rectangles overlap (~one
N=512 "slot" of ~306ns for the whole pack). Per kc iteration (128 k
positions, q block 512 wide):

  - scores pack (1 slot): the two heads' S^T matmuls (K=64 contraction)
    row-pack into opposite halves of the array, writing the two banks of
    one [128, 2(head), 512] PSUM tile (double buffered).
  - one 1024-col exp on ScalarE covers both heads (scale folded in),
    bf16 out; ScalarE is the target pacing engine (~1.05us/kc).
  - AV pack (1 slot): O^T[h0] (cols 0-63) and O^T[h1] (cols 64-127)
    col-pack into ONE shared PSUM bank; kc=0 uses start=True on the first
    tile only (clears the bank) and start=False on the second (overwrite-
    where-unset per has_written semantics).
  - denominator pack (0.5 slot): every second kc, four M=1 col-tiles
    (ones[128,1] lhsT at col positions 0/32/64/96) accumulate
    sum_k exp for (kc-1,kc)x(h0,h1) into one [128,512] PSUM bank; the
    finalize step adds the two partial rows per head.
  - projections for later head pairs are emitted as single-matmul filler
    closures interleaved into the loop.

PSUM budget (8 banks): scores 2x[128,2,512]=4, O accum [128,512]x2bufs=2,
den [128,512]=1, projection staging [128,512]=1.
"""
import numpy as np
import ml_dtypes
from contextlib import ExitStack

import concourse.tile as tile
import concourse.mybir as mybir
from concourse import bacc
from concourse.bass_utils import run_bass_kernel_spmd

P = 128
DH = 64
BF = mybir.dt.bfloat16
F32 = mybir.dt.float32


def build_attention(S=2048, D=1024, HPC=8, loop_n=1, ablate=(), pbufs=4, pops=2,
                    early_proj=False):
    """Build the per-core SPMD program. HPC = heads per core (even).

    loop_n > 1 wraps the whole body in a hardware loop (for timing)."""
    DC = D // P        # D chunks of 128
    KC = S // P        # k chunks of 128
    NQ = S // 512      # q blocks of 512
    HP = HPC // 2      # head pairs
    CW = HPC * DH      # core output width
    SCALE = 1.0 / float(np.sqrt(DH))

    nc = bacc.Bacc("TRN2")
    xq = nc.dram_tensor("xq", [DC, P, S], BF, kind="ExternalInput")
    xk = nc.dram_tensor("xk", [DC, P, S], BF, kind="ExternalInput")
    xv = nc.dram_tensor("xv", [DC, P, S], BF, kind="ExternalInput")
    wq = nc.dram_tensor("wq", [DC, P, CW], BF, kind="ExternalInput")
    wk = nc.dram_tensor("wk", [DC, P, CW], BF, kind="ExternalInput")
    wv = nc.dram_tensor("wv", [DC, P, CW], BF, kind="ExternalInput")
    out = nc.dram_tensor("out", [HPC, DH, S], F32, kind="ExternalOutput")

    with tile.TileContext(nc) as tc, ExitStack() as ctx:
        xpool = ctx.enter_context(tc.tile_pool(name="x", bufs=1))
        wpool = ctx.enter_context(tc.tile_pool(name="w", bufs=1))
        vpool = ctx.enter_context(tc.tile_pool(name="v", bufs=1))
        qkpool = ctx.enter_context(
            tc.tile_pool(name="qk", bufs=4 if early_proj else 2))
        ppool = ctx.enter_context(tc.tile_pool(name="p", bufs=pbufs))
        opool = ctx.enter_context(tc.tile_pool(name="one", bufs=1))
        ostag = ctx.enter_context(tc.tile_pool(name="ost", bufs=3))
        outp = ctx.enter_context(tc.tile_pool(name="outp", bufs=3))
        rpool = ctx.enter_context(tc.tile_pool(name="r", bufs=2))
        ps_s = ctx.enter_context(tc.tile_pool(name="ps_s", bufs=2, space="PSUM"))
        ps_o = ctx.enter_context(tc.tile_pool(name="ps_o", bufs=2, space="PSUM"))
        ps_d = ctx.enter_context(tc.tile_pool(name="ps_d", bufs=1, space="PSUM"))
        ps_m = ctx.enter_context(tc.tile_pool(name="ps_m", bufs=1, space="PSUM"))

        xs, ws = {}, {}
        vt = None
        ones = None

        def emit_loads():
            nonlocal vt, ones
            for name, dram in [("q", wq), ("k", wk), ("v", wv)]:
                t = wpool.tile([P, DC, CW], BF, tag="w" + name, name="w" + name)
                for dc in range(DC):
                    nc.sync.dma_start(t[:, dc, :], dram[dc])
                ws[name] = t
            for name, dram in [("q", xq), ("k", xk), ("v", xv)]:
                t = xpool.tile([P, DC, S], BF, tag="x" + name, name="x" + name)
                for dc in range(DC):
                    nc.sync.dma_start(t[:, dc, :], dram[dc])
                xs[name] = t
            vt = vpool.tile([P, KC, HPC, DH], BF, tag="V", name="vt")
            ones = opool.tile([P, 1], BF, tag="ones", name="ones")
            nc.any.memset(ones[:], 1.0)

        def proj_v_kc(kc):
            pv = ps_m.tile([P, 512], F32, tag="proj", name="pv")[:, :CW]
            for dc in range(DC):
                nc.tensor.matmul(
                    pv,
                    xs["v"][:, dc, kc * P : (kc + 1) * P],
                    ws["v"][:, dc, :],
                    start=(dc == 0),
                    stop=(dc == DC - 1),
                )
            nc.vector.tensor_copy(
                vt[:, kc, :, :],
                pv.rearrange("p (h d) -> p h d", d=DH),
            )

        def proj_qk_chunk(t, which, hp, qb):
            pp = ps_m.tile([P, 512], F32, tag="proj")
            for dc in range(DC):
                nc.tensor.matmul(
                    pp[:],
                    ws[which][:, dc, hp * P : (hp + 1) * P],
                    xs[which][:, dc, qb * 512 : (qb + 1) * 512],
                    start=(dc == 0),
                    stop=(dc == DC - 1),
                )
            nc.vector.tensor_copy(t[:, qb * 512 : (qb + 1) * 512], pp[:])

        def new_qk(which):
            return qkpool.tile([P, S], BF, tag=which, name=which + "t")

        def proj_qk_fillers(t, which, hp):
            """Projection of one tensor for head pair hp as a list of small
            filler closures (one matmul each; the last also evacuates)."""
            fillers = []
            for qb in range(NQ):
                state = {}

                def mk(dc, qb=qb, state=state):
                    def f():
                        if dc == 0:
                            state["pp"] = ps_m.tile([P, 512], F32, tag="proj",
                                                    name="pp")
                        pp = state["pp"]
                        nc.tensor.matmul(
                            pp[:],
                            ws[which][:, dc, hp * P : (hp + 1) * P],
                            xs[which][:, dc, qb * 512 : (qb + 1) * 512],
                            start=(dc == 0),
                            stop=(dc == DC - 1),
                        )
                        if dc == DC - 1:
                            nc.vector.tensor_copy(
                                t[:, qb * 512 : (qb + 1) * 512], pp[:])
                    return f

                fillers += [mk(d) for d in range(DC)]
            return fillers

        def finalize_fillers(osb, den_sb, hp, qb):
            """Normalize + store one finished q block: one closure per
            (head, op). den_sb holds the 4 denominator partial rows."""
            fillers = []
            if "nofin" in ablate or "noden" in ablate:
                return []
            for h in (0, 1):
                ch = hp * 2 + h
                state = {}

                def mv(h=h, state=state, den_sb=den_sb):
                    # den_sb rows: 0=(even kc,h0), 32=(even kc,h1),
                    #              64=(odd kc,h0), 96=(odd kc,h1).
                    # DMA the two partial rows for head h to partition 0.
                    dA = rpool.tile([1, 512], F32, tag="dA", name="dA")
                    dB = rpool.tile([1, 512], F32, tag="dB", name="dB")
                    nc.sync.dma_start(dA[0:1, :], den_sb[h * 32 : h * 32 + 1, :])
                    nc.sync.dma_start(
                        dB[0:1, :], den_sb[64 + h * 32 : 64 + h * 32 + 1, :])
                    state["dA"], state["dB"] = dA, dB

                def rec(state=state):
                    dsum = rpool.tile([1, 512], F32, tag="ds", name="dsum")
                    nc.vector.tensor_tensor(
                        dsum[:], state["dA"][:], state["dB"][:],
                        mybir.AluOpType.add)
                    rsb = rpool.tile([1, 512], F32, tag="rc", name="rsb")
                    nc.vector.reciprocal(rsb[:], dsum[:])
                    state["rsb"] = rsb

                def bc(state=state):
                    # broadcast to all 128 partitions so the head-h slice is
                    # partition-aligned with osb's slice
                    rbc = rpool.tile([P, 512], F32, tag="rbc", name="rbc")
                    nc.gpsimd.partition_broadcast(rbc[:], state["rsb"][0:1, :])
                    state["rbc"] = rbc

                def norm(ch=ch, qb=qb, h=h, osb=osb, state=state):
                    ot = outp.tile([P, 512], F32, tag="ot", name="ot")
                    sl = slice(h * DH, (h + 1) * DH)
                    nc.vector.tensor_tensor(
                        ot[sl, :], osb[sl, :], state["rbc"][sl, :],
                        mybir.AluOpType.mult)
                    nc.sync.dma_start(
                        out[ch, :, qb * 512 : (qb + 1) * 512], ot[sl, :])

                fillers += [mv, rec, bc, norm]
            return fillers

        def attn_block(hp, qb, qt, kt, first, proj_q, fin_q):
            """Attention for head pair hp, q block qb (512 wide)."""
            while len(fin_q) > 8:
                fin_q.pop(0)()
            o_ps = ps_o.tile([P, 512], F32, tag="O", name="o_ps")
            den_ps = ps_d.tile([P, 512], F32, tag="den", name="den_ps")

            def emit_scores(kc):
                s = ps_s.tile([P, 2, 512], F32, tag="S", name="s")
                for h in (0, 1):
                    # K=64 contraction; the two heads' lhsT/rhs live on
                    # opposite partition halves -> row-packed in the PE array
                    nc.tensor.matmul(
                        s[:, h, :],
                        kt[h * DH : (h + 1) * DH, kc * P : (kc + 1) * P],
                        qt[h * DH : (h + 1) * DH, qb * 512 : (qb + 1) * 512],
                        start=True,
                        stop=True,
                    )
                return s

            def emit_den_pack(den_ps, pts, kc0, first, last):
                for j, (kcd, h) in enumerate(
                        [(kc0, 0), (kc0, 1), (kc0 + 1, 0), (kc0 + 1, 1)]):
                    nc.tensor.matmul(
                        den_ps[j * 32 : j * 32 + 1, :],
                        ones[:],
                        pts[kcd][:, h, :],
                        start=first,
                        stop=last,
                        tile_position=(0, j * 32),
                        skip_group_check=(j > 0),
                    )

            s_cur = emit_scores(0)
            pts = {}
            for kc in range(KC):
                pt = ppool.tile([P, 2, 512], BF, tag="pt")
                pts[kc] = pt
                nc.scalar.activation(
                    pt[:], s_cur[:], mybir.ActivationFunctionType.Exp,
                    scale=SCALE)
                if kc + 1 < KC:
                    s_cur = emit_scores(kc + 1)
                # V projection just-in-time during the first attn pass
                if first and qb == 0:
                    proj_v_kc(kc)
                # AV pack: both heads col-packed into one bank
                for h in (0, 1):
                    ch = hp * 2 + h
                    nc.tensor.matmul(
                        o_ps[h * DH : (h + 1) * DH, :],
                        vt[:, kc, ch, :],
                        pt[:, h, :],
                        start=(kc == 0),
                        stop=(kc == KC - 1),
                        skip_group_check=(h == 1),
                    )
                # denominator pack: 4 M=1 col-tiles for (kc-2, kc-1) x (h0, h1).
                # Window trails by one kc so all four pts are already
                # available -> the scheduler keeps the pack contiguous.
                if kc % 2 == 0 and kc >= 2 and "noden" not in ablate:
                    emit_den_pack(den_ps, pts, kc - 2, first=(kc == 2),
                                  last=False)
                    if kc - 3 in pts:
                        del pts[kc - 2], pts[kc - 3]
                # interleave deferred work while ScalarE paces the loop
                if not (first and qb == 0) and kc < KC - 1:
                    budget = pops
                    while budget and (proj_q or fin_q):
                        (proj_q or fin_q).pop(0)()
                        budget -= 1
            if "noden" in ablate:
                osb = ostag.tile([P, 512], F32, tag="osb")
                nc.vector.tensor_copy(osb[:], o_ps[:])
                return osb, None
            emit_den_pack(den_ps, pts, KC - 2, first=False, last=True)
            # evacuate O and den PSUM now; normalize runs as fillers later
            osb = ostag.tile([P, 512], F32, tag="osb")
            nc.vector.tensor_copy(osb[:], o_ps[:])
            den_sb = ostag.tile([97, 512], F32, tag="densb")
            for j in range(4):
                nc.vector.tensor_copy(
                    den_sb[j * 32 : j * 32 + 1, :], den_ps[j * 32 : j * 32 + 1, :])
            return osb, den_sb

        def emit_body():
            emit_loads()
            qt = new_qk("q")
            kt = new_qk("k")
            for qb in range(NQ):
                proj_qk_chunk(qt, "q", 0, qb)
                proj_qk_chunk(kt, "k", 0, qb)
            proj_q, fin_q = [], []
            nxt = {}
            for hp in range(HP):
                if early_proj:
                    if hp == 0:
                        for h2 in range(1, HP):
                            qt2, kt2 = new_qk("q"), new_qk("k")
                            nxt[h2] = (qt2, kt2)
                            proj_q += proj_qk_fillers(qt2, "q", h2)
                            proj_q += proj_qk_fillers(kt2, "k", h2)
                elif hp + 1 < HP:
                    qt_next = new_qk("q")
                    kt_next = new_qk("k")
                    nxt[hp + 1] = (qt_next, kt_next)
                    proj_q += proj_qk_fillers(qt_next, "q", hp + 1)
                    proj_q += proj_qk_fillers(kt_next, "k", hp + 1)
                for qb in range(NQ):
                    osb, den_sb = attn_block(hp, qb, qt, kt, first=(hp == 0),
                                             proj_q=proj_q, fin_q=fin_q)
                    fin_q += finalize_fillers(osb, den_sb, hp, qb)
                if not early_proj:
                    # the next head pair's projections must be fully emitted
                    # before its attention reads them
                    for f in proj_q:
                        f()
                    proj_q = []
                elif hp + 1 < HP:
                    # ensure the next pair's projections are fully emitted
                    while proj_q and len(proj_q) > (HP - 2 - hp) * 2 * NQ * DC:
                        proj_q.pop(0)()
                if hp + 1 < HP:
                    qt, kt = nxt[hp + 1]
            for f in fin_q:
                f()

        if loop_n > 1:
            with tc.For_i(0, loop_n, 1):
                emit_body()
        else:
            emit_body()

    nc.compile()
    return nc


_NC_CACHE = {}


def _get_nc(S, D, HPC):
    key = (S, D, HPC)
    if key not in _NC_CACHE:
        _NC_CACHE[key] = build_attention(S, D, HPC)
    return _NC_CACHE[key]


def _prep_core_inputs(q_seq, k_seq, v_seq, WQ, WK, WV, b, hg, HPC, D):
    """Host-side shard prep for core (batch b, head group hg)."""
    DC = D // P
    CW = HPC * DH
    bf16 = ml_dtypes.bfloat16

    def xt(x):  # [S, D] -> [DC, P, S] (D-major transpose)
        return np.ascontiguousarray(x.T.reshape(DC, P, -1)).astype(bf16)

    def wslice(w):  # [D, out] -> [DC, P, CW]
        return np.ascontiguousarray(
            w[:, hg * CW : (hg + 1) * CW].reshape(DC, P, CW)
        ).astype(bf16)

    return {
        "xq": xt(q_seq[b]),
        "xk": xt(k_seq[b]),
        "xv": xt(v_seq[b]),
        "wq": wslice(WQ),
        "wk": wslice(WK),
        "wv": wslice(WV),
    }


def kernel(q_seq, k_seq, v_seq, WQ, WK, WV, _trace=False):
    q_seq = np.asarray(q_seq, dtype=np.float32)
    k_seq = np.asarray(k_seq, dtype=np.float32)
    v_seq = np.asarray(v_seq, dtype=np.float32)
    WQ = np.asarray(WQ, dtype=np.float32)
    WK = np.asarray(WK, dtype=np.float32)
    WV = np.asarray(WV, dtype=np.float32)

    B, S, D = q_seq.shape
    NB_HEAD = WQ.shape[1] // DH
    n_cores = 8
    groups_per_batch = n_cores // B          # 2 head groups
    HPC = NB_HEAD // groups_per_batch        # 8 heads per core
    CW = HPC * DH

    nc = _get_nc(S, D, HPC)

    in_maps = []
    for core in range(n_cores):
        b, hg = core // groups_per_batch, core % groups_per_batch
        in_maps.append(_prep_core_inputs(q_seq, k_seq, v_seq, WQ, WK, WV, b, hg, HPC, D))

    res = run_bass_kernel_spmd(
        nc, in_maps, core_ids=list(range(n_cores)), trace=_trace,
        **({"trace_cores": [0], } if _trace else {}),
    )
    if _trace:
        print(f"HW exec time: {res.exec_time_ns} ns")
        if res.instructions_and_trace:
            print("trace:", res.instructions_and_trace[1])

    out = np.empty((B, S, NB_HEAD * DH), dtype=np.float32)
    for core in range(n_cores):
        b, hg = core // groups_per_batch, core % groups_per_batch
        # device output is O^T per head: [HPC, DH, S] -> [S, HPC*DH]
        ot = res.results[core]["out"]
        out[b, :, hg * CW : (hg + 1) * CW] = (
            ot.transpose(2, 0, 1).reshape(S, CW)
        )
    return out



# revision 22
# speedup vs baseline: 2.1448x; 2.1448x over previous
"""Multi-head attention Bass kernel for Trainium2, SPMD over 8 NeuronCores.

Problem: B=4, S=2048, D=1024, 16 heads x 64. Sharding: core = (batch b, head-group hg)
with b in 0..3, hg in 0..1 -> each core computes 8 heads of one batch.

Design (cost-model driven, fp16 end-to-end):
  - ScalarE's exp is the hard floor: 256 activations of [128(k), 2(head),
    512(q)] PSUM fp32 -> p fp16, ~1.04us each = ~266us. Everything else is
    arranged to hide under it.
  - scores (PE): per (kc, head) one fp16 matmul K=64 -> s[k, q] in PSUM.
  - AV is Q-MAJOR: O[q, dh] accumulates with M=128 q-positions on PSUM
    partitions and only N=66 columns (64 dh + a ones column that picks up
    the softmax denominator for free + 1 pad for 8B alignment). lhsT is
    the p tile (stationary), rhs is V-augmented [k, 66]. Cost: 66 cycles
    per (kc, head, q-128-chunk) -> ~58us total, half of the k-major form,
    and the denominator lands per-partition-aligned with q so normalize is
    reciprocal + tensor_scalar_mul per chunk - no cross-partition traffic.
  - Four accumulation chains share each PSUM bank; only the chain writing
    first uses start=True (clears the whole bank's has_written bits), the
    others overwrite-where-unset.
  - All deferred work (V projection chunks, AV+finalize, later Q/K
    projections) sits in queues annotated with the earliest "slot" (ACT
    count) at which its input DMA will have landed, so a not-yet-ready
    instruction never enters the PE FIFO ahead of the scores matmuls that
    feed ScalarE. AV closures additionally gate on their V chunk being
    emitted; finalize closures ride the same queue so o_ps frees in order.
  - inputs stream as column-sliced DMAs in consumption order (the DMA
    engine pool is serial in the cost model): wq, wk, xk0, xq0, xk1, xv0,
    xk2, xv1, xk3, xq1, xv2, xv3, xq2, xq3.

PSUM (8 banks): scores 2bufs x [128,2,512] f32 = 4, O accum 2 (2qc x 2h x 66
x 2 banks), projection staging 2.
"""
import numpy as np
import ml_dtypes
from contextlib import ExitStack

import concourse.tile as tile
import concourse.mybir as mybir
from concourse import bacc
from concourse.bass_utils import run_bass_kernel_spmd

P = 128
DH = 64
F16 = mybir.dt.float16
F32 = mybir.dt.float32

AV_START = 2          # earliest kc for AV draining (first block)
AV_RATE = 4           # max AV/finalize closures drained per kc
V_RATE = 3            # max V-projection closures drained per kc
# earliest global slot for V chunk group c//4 (when xv quarter c//4 landed)
V_SLOTS = (12, 17, 20, 23)
PQ_RATE = 3           # max projection closures drained per kc


def build_attention(S=2048, D=1024, HPC=8, loop_n=1, pops=PQ_RATE):
    """Build the per-core SPMD program. HPC = heads per core (even).

    loop_n > 1 wraps the whole body in a hardware loop (for timing)."""
    DC = D // P        # D chunks of 128
    KC = S // P        # k chunks of 128
    NQ = S // 512      # q blocks of 512
    HP = HPC // 2      # head pairs
    CW = HPC * DH      # core output width
    ACT_SCALE = 1.0 / float(np.sqrt(DH))

    nc = bacc.Bacc("TRN2")
    xq = nc.dram_tensor("xq", [P, DC, S], F16, kind="ExternalInput")
    xk = nc.dram_tensor("xk", [P, DC, S], F16, kind="ExternalInput")
    xv = nc.dram_tensor("xv", [P, DC, S], F16, kind="ExternalInput")
    wq = nc.dram_tensor("wq", [P, DC, CW], F16, kind="ExternalInput")
    wk = nc.dram_tensor("wk", [P, DC, CW], F16, kind="ExternalInput")
    wv = nc.dram_tensor("wv", [P, DC, CW], F16, kind="ExternalInput")
    out = nc.dram_tensor("out", [HPC, P, NQ, 4, DH], F32, kind="ExternalOutput")

    with tile.TileContext(nc) as tc, ExitStack() as ctx:
        xpool = ctx.enter_context(tc.tile_pool(name="x", bufs=1))
        wpool = ctx.enter_context(tc.tile_pool(name="w", bufs=1))
        vpool = ctx.enter_context(tc.tile_pool(name="v", bufs=1))
        qkpool = ctx.enter_context(tc.tile_pool(name="qk", bufs=3))
        ppool = ctx.enter_context(tc.tile_pool(name="p", bufs=14))
        rpool = ctx.enter_context(tc.tile_pool(name="r", bufs=4))
        opool = ctx.enter_context(tc.tile_pool(name="ob", bufs=2))
        otpool = ctx.enter_context(tc.tile_pool(name="ot", bufs=2))
        ps_s = ctx.enter_context(tc.tile_pool(name="ps_s", bufs=2, space="PSUM"))
        ps_o = ctx.enter_context(tc.tile_pool(name="ps_o", bufs=1, space="PSUM"))
        ps_m = ctx.enter_context(tc.tile_pool(name="ps_m", bufs=2, space="PSUM"))

        xs, ws = {}, {}
        vta = None
        slot = [0]           # global ACT counter
        vta_done = [False] * KC

        def emit_loads():
            nonlocal vta
            for name in ("q", "k", "v"):
                ws[name] = wpool.tile([P, DC, CW], F16, tag="w" + name,
                                      name="w" + name)
                xs[name] = xpool.tile([P, DC, S], F16, tag="x" + name,
                                      name="x" + name)

            def ld(t, dram, c0, c1):
                nc.sync.dma_start(t[:, :, c0:c1], dram[:, :, c0:c1])

            # DMA order = consumption order (DMA engine pool is serial).
            # hp0's weight columns first; the rest of W after the k/v bulk.
            nc.sync.dma_start(ws["q"][:, :, 0:P], wq[:, :, 0:P])
            nc.sync.dma_start(ws["k"][:, :, 0:P], wk[:, :, 0:P])
            ld(xs["q"], xq, 0, 512)
            ld(xs["k"], xk, 0, 512)
            ld(xs["k"], xk, 512, 1024)
            ld(xs["k"], xk, 1024, 1536)
            nc.sync.dma_start(ws["v"][:], wv[:])
            ld(xs["k"], xk, 1536, 2048)
            ld(xs["v"], xv, 0, 512)
            ld(xs["q"], xq, 512, 1024)
            ld(xs["v"], xv, 512, 1024)
            ld(xs["v"], xv, 1024, 1536)
            ld(xs["v"], xv, 1536, 2048)
            nc.sync.dma_start(ws["q"][:, :, P:CW], wq[:, :, P:CW])
            nc.sync.dma_start(ws["k"][:, :, P:CW], wk[:, :, P:CW])
            ld(xs["q"], xq, 1024, 1536)
            ld(xs["q"], xq, 1536, 2048)
            # V-augmented rhs: [kpos, kc, ch, 66] = V | 1.0 | 0 pad
            vta = vpool.tile([P, KC, HPC, 66], F16, tag="V", name="vta")
            nc.vector.memset(vta[:, :, :, 64], 1.0)
            nc.vector.memset(vta[:, :, :, 65], 0.0)

        def v_closures(kc, min_slot):
            pstate = {}

            def mk(dc):
                def f():
                    if dc == 0:
                        pstate["pv"] = ps_m.tile([P, 512], F32,
                                                 tag="proj", name="pv")
                    nc.tensor.matmul(
                        pstate["pv"][:, :CW],
                        xs["v"][:, dc, kc * P : (kc + 1) * P],
                        ws["v"][:, dc, :],
                        start=(dc == 0),
                        stop=(dc == DC - 1),
                    )
                    if dc == DC - 1:
                        nc.vector.tensor_copy(
                            vta[:, kc, :, 0:DH],
                            pstate["pv"][:, :CW].rearrange(
                                "p (h d) -> p h d", d=DH),
                        )
                        vta_done[kc] = True
                return f

            return [(min_slot, mk(d)) for d in range(DC)]

        def new_qk(which):
            return qkpool.tile([P, S], F16, tag=which, name=which + "t")

        def proj_qk_chunk(t, which, hp, qb):
            pp = ps_m.tile([P, 512], F32, tag="proj", name="pp")
            for dc in range(DC):
                nc.tensor.matmul(
                    pp[:],
                    ws[which][:, dc, hp * P : (hp + 1) * P],
                    xs[which][:, dc, qb * 512 : (qb + 1) * 512],
                    start=(dc == 0),
                    stop=(dc == DC - 1),
                )
            nc.vector.tensor_copy(t[:, qb * 512 : (qb + 1) * 512], pp[:])

        def chunk_closures(t, which, hp, qb, min_slot, c0=0, c1=512):
            """(min_slot, closure) items: one per matmul; last also
            evacuates. c0:c1 select columns within the 512-wide chunk."""
            pstate = {}
            w = c1 - c0

            def mk(dc):
                def f():
                    if dc == 0:
                        pstate["pp"] = ps_m.tile([P, 512], F32,
                                                 tag="proj", name="pp")
                    nc.tensor.matmul(
                        pstate["pp"][:, 0:w],
                        ws[which][:, dc, hp * P : (hp + 1) * P],
                        xs[which][:, dc, qb * 512 + c0 : qb * 512 + c1],
                        start=(dc == 0),
                        stop=(dc == DC - 1),
                    )
                    if dc == DC - 1:
                        nc.vector.tensor_copy(
                            t[:, qb * 512 + c0 : qb * 512 + c1],
                            pstate["pp"][:, 0:w])
                return f

            return [(min_slot, mk(d)) for d in range(DC)]

        def drain(q, budget, gate=None):
            while budget and q:
                head = q[0]
                if head[0] is not None and head[0] > slot[0]:
                    break
                if gate is not None and not gate(head):
                    break
                q.pop(0)[1]()
                budget -= 1

        def attn_block(hp, qb, qt, kt, proj_q, v_q, av_q, kt_done, q0_q, first_hp=False, last=False):
            # o banks: [128(q), 2(qc half), 2(head), 66]; qc 0,1 -> bank A,
            # qc 2,3 -> bank B
            o_ps = [ps_o.tile([P, 2, 2, 66], F32, tag=f"O{i}", name=f"o{i}")
                    for i in (0, 1)]

            def emit_scores(kc):
                s = ps_s.tile([P, 2, 512], F32, tag="S", name="s")
                for h in (0, 1):
                    nc.tensor.matmul(
                        s[:, h, :],
                        kt[h * DH : (h + 1) * DH, kc * P : (kc + 1) * P],
                        qt[h * DH : (h + 1) * DH, qb * 512 : (qb + 1) * 512],
                        start=True,
                        stop=True,
                    )
                return s

            def mk_av(kc, pt):
                def f():
                    for qc in range(4):
                        for h in (0, 1):
                            nc.tensor.matmul(
                                o_ps[qc // 2][:, qc % 2, h, :],
                                pt[:, h, qc * P : (qc + 1) * P],
                                vta[:, kc, hp * 2 + h, :],
                                start=(kc == 0 and qc % 2 == 0 and h == 0),
                                stop=(kc == KC - 1),
                                skip_group_check=(qc + h > 0),
                            )
                return f

            def finalize():
                ot = otpool.tile([P, 4, 2, DH], F32, tag="ot", name="ot")
                for i in (0, 1):
                    osb = opool.tile([P, 2, 2, 66], F32, tag="osb", name="osb")
                    nc.vector.tensor_copy(osb[:], o_ps[i][:])
                    for j in (0, 1):
                        for h in (0, 1):
                            rt = rpool.tile([P, 1], F32, tag="rt", name="rt")
                            nc.vector.reciprocal(rt[:], osb[:, j, h, 64:65])
                            nc.vector.tensor_scalar_mul(
                                ot[:, 2 * i + j, h, :], osb[:, j, h, 0:DH],
                                rt[:, 0:1])
                for h in (0, 1):
                    ch = hp * 2 + h
                    nc.sync.dma_start(out[ch, :, qb, :, :], ot[:, :, h, :])

            def gate_av(head):
                kc = head[2]
                return kc is None or vta_done[kc]

            s_cur = emit_scores(0)
            for kc in range(KC):
                pt = ppool.tile([P, 2, 512], F16, tag="p", name="pt")
                nc.scalar.activation(
                    pt[:], s_cur[:],
                    mybir.ActivationFunctionType.Exp,
                    scale=ACT_SCALE)
                slot[0] += 1
                if kc + 1 < KC:
                    if first_hp and qb == 0:
                        # force-drain deferred kt work until the columns the
                        # next scores matmul reads have been projected
                        while kt_done[0] < (kc + 2) * P and proj_q:
                            assert proj_q[0][2] == -1
                            proj_q.pop(0)[1]()
                    s_cur = emit_scores(kc + 1)
                drain(v_q, V_RATE)
                av_q.append((None, mk_av(kc, pt), kc))
                drain(av_q, len(av_q) if last else AV_RATE, gate=gate_av)
                drain(q0_q, 2)
                drain(proj_q, pops)
            av_q.append((None, finalize, None))

        def emit_body():
            emit_loads()
            qt = new_qk("q")
            kt = new_qk("k")
            # warm the PE p-state during the input-DMA wait: dummy
            # matmuls over the zeroed vta keep the ramp model at full speed
            # for the first real projections
            oc = vta[:, 0, :, 64:66]   # [P, HPC, 2] initialized slice
            for i in range(50):
                wp = ps_m.tile([P, 512], F32, tag="proj", name="wp")
                nc.tensor.matmul(
                    wp[0:1, 0:256],
                    oc[:, 0, 0:1],
                    oc.to_broadcast((P, HPC, 2, 16)),
                    start=True, stop=True)
            # prologue: just enough projection for the first scores:
            # Q chunk 0 (xq0 lands first), then K chunk 0 cols 0:256.
            proj_qk_chunk(qt, "q", 0, 0)
            for _s, f in chunk_closures(kt, "k", 0, 0, 0, 0, 256):
                f()
            # deferred, force-drained ahead of the scores that read them
            # (tag -1 entries carry kt columns; kt_cols tracks progress)
            proj_q = []
            kt_done = [256]

            def mark(cols):
                def g():
                    kt_done[0] = cols
                return g

            proj_q += [(0, f, -1) for _s, f in
                       chunk_closures(kt, "k", 0, 0, 0, 256, 512)]
            proj_q.append((0, mark(512), -1))
            proj_q += [(0, f, -1) for _s, f in
                       chunk_closures(kt, "k", 0, 1, 0)]
            proj_q.append((0, mark(1024), -1))
            proj_q += [(4, f, -1) for _s, f in
                       chunk_closures(kt, "k", 0, 2, 4)]
            proj_q.append((4, mark(1536), -1))
            proj_q += [(8, f, -1) for _s, f in
                       chunk_closures(kt, "k", 0, 3, 8)]
            proj_q.append((8, mark(2048), -1))
            q0_q = []
            for qb, ms in ((1, 15), (2, 29), (3, 32)):
                q0_q += [(s0, f, qb) for s0, f in
                         chunk_closures(qt, "q", 0, qb, ms)]
            v_q = []
            for kc in range(KC):
                v_q += v_closures(kc, V_SLOTS[kc // 4] + 2 * (kc % 4))
            av_q = []

            # prefetch queues for hp 1..3, tagged with their hp so the
            # boundary flush can force-complete exactly what's needed
            qts = {0: (qt, kt)}
            for hpn in range(1, HP):
                base = (27, 27, 72, 136)[hpn]
                qts[hpn] = (new_qk("q"), new_qk("k"))
                for qb in range(NQ):
                    proj_q += [(max(s0, base), f, hpn) for s0, f in
                               chunk_closures(qts[hpn][1], "k", hpn, qb, 0)]
                for qb in range(NQ):
                    proj_q += [(max(s0, base, 26), f, hpn) for s0, f in
                               chunk_closures(qts[hpn][0], "q", hpn, qb, 0)]
            for hp in range(HP):
                qt, kt = qts[hp]
                for qb in range(NQ):
                    if hp == 0:
                        # hp0's qt chunk qb must be fully projected before
                        # this block's scores read it
                        while q0_q and q0_q[0][2] <= qb:
                            q0_q.pop(0)[1]()
                    attn_block(hp, qb, qt, kt, proj_q, v_q, av_q, kt_done,
                               q0_q if hp == 0 else [],
                               first_hp=(hp == 0),
                               last=(hp == HP - 1 and qb == NQ - 1))
                # next head pair's projections must be fully emitted before
                # its attention reads them
                if hp + 1 < HP:
                    while proj_q and proj_q[0][2] <= hp + 1:
                        proj_q.pop(0)[1]()
            while v_q:
                v_q.pop(0)[1]()
            while av_q:
                av_q.pop(0)[1]()

        if loop_n > 1:
            with tc.For_i(0, loop_n, 1):
                emit_body()
        else:
            emit_body()

    nc.compile()
    return nc


_NC_CACHE = {}


def _get_nc(S, D, HPC):
    key = (S, D, HPC)
    if key not in _NC_CACHE:
        _NC_CACHE[key] = build_attention(S, D, HPC)
    return _NC_CACHE[key]


def _prep_batch_x(q_seq, k_seq, v_seq, b, D):
    """Per-batch fp16 x^T shards (shared by the 2 head-group cores)."""
    DC = D // P

    def xt(x):  # [S, D] -> [P, DC, S]
        return np.ascontiguousarray(
            x.T.reshape(DC, P, -1).transpose(1, 0, 2)).astype(np.float16)

    return {"xq": xt(q_seq[b]), "xk": xt(k_seq[b]), "xv": xt(v_seq[b])}


def _prep_w(WQ, WK, WV, hg, HPC, D):
    """Per-head-group fp16 weight shards."""
    DC = D // P
    CW = HPC * DH

    def wslice(w):  # [D, out] -> [P, DC, CW]
        return np.ascontiguousarray(
            w[:, hg * CW : (hg + 1) * CW]
            .reshape(DC, P, CW).transpose(1, 0, 2)).astype(np.float16)

    return {"wq": wslice(WQ), "wk": wslice(WK), "wv": wslice(WV)}


def _prep_core_inputs(q_seq, k_seq, v_seq, WQ, WK, WV, b, hg, HPC, D):
    """Host-side shard prep for core (batch b, head group hg)."""
    m = _prep_batch_x(q_seq, k_seq, v_seq, b, D)
    m.update(_prep_w(WQ, WK, WV, hg, HPC, D))
    return m


def kernel(q_seq, k_seq, v_seq, WQ, WK, WV, _trace=False):
    q_seq = np.asarray(q_seq, dtype=np.float32)
    k_seq = np.asarray(k_seq, dtype=np.float32)
    v_seq = np.asarray(v_seq, dtype=np.float32)
    WQ = np.asarray(WQ, dtype=np.float32)
    WK = np.asarray(WK, dtype=np.float32)
    WV = np.asarray(WV, dtype=np.float32)

    B, S, D = q_seq.shape
    NB_HEAD = WQ.shape[1] // DH
    n_cores = 8
    groups_per_batch = n_cores // B          # 2 head groups
    HPC = NB_HEAD // groups_per_batch        # 8 heads per core
    CW = HPC * DH

    nc = _get_nc(S, D, HPC)

    xmaps = {b: _prep_batch_x(q_seq, k_seq, v_seq, b, D) for b in range(B)}
    wmaps = {hg: _prep_w(WQ, WK, WV, hg, HPC, D) for hg in range(groups_per_batch)}
    in_maps = []
    for core in range(n_cores):
        b, hg = core // groups_per_batch, core % groups_per_batch
        in_maps.append({**xmaps[b], **wmaps[hg]})

    res = run_bass_kernel_spmd(
        nc, in_maps, core_ids=list(range(n_cores)), trace=_trace,
        **({"trace_cores": [0], } if _trace else {}),
    )
    if _trace:
        print(f"HW exec time: {res.exec_time_ns} ns")
        if res.instructions_and_trace:
            print("trace:", res.instructions_and_trace[1])

    out = np.empty((B, S, NB_HEAD * DH), dtype=np.float32)
    for core in range(n_cores):
        b, hg = core // groups_per_batch, core % groups_per_batch
        # device output: [HPC, P, NQ, 4, DH]; q = qb*512 + qc*128 + p
        ot = res.results[core]["out"]
        ot = ot.transpose(2, 3, 1, 0, 4).reshape(S, CW)
        out[b, :, hg * CW : (hg + 1) * CW] = ot
    return out


# revision 27
# speedup vs baseline: 2.2421x; 1.0454x over previous
"""Multi-head attention Bass kernel for Trainium2, SPMD over 8 NeuronCores.

Problem: B=4, S=2048, D=1024, 16 heads x 64. Sharding: core = (batch b, head-group hg)
with b in 0..3, hg in 0..1 -> each core computes 8 heads of one batch.

Design (cost-model driven, fp16 end-to-end):
  - ScalarE's exp is the hard floor: 256 activations of [128(k), 2(head),
    512(q)] PSUM fp32 -> p fp16, ~1.04us each = ~266us. Everything else is
    arranged to hide under it.
  - scores (PE): per (kc, head) one fp16 matmul K=64 -> s[k, q] in PSUM.
  - AV is Q-MAJOR: O[q, dh] accumulates with M=128 q-positions on PSUM
    partitions and only N=66 columns (64 dh + a ones column that picks up
    the softmax denominator for free + 1 pad for 8B alignment). lhsT is
    the p tile (stationary), rhs is V-augmented [k, 66]. Cost: 66 cycles
    per (kc, head, q-128-chunk) -> ~58us total, half of the k-major form,
    and the denominator lands per-partition-aligned with q so normalize is
    reciprocal + tensor_scalar_mul per chunk - no cross-partition traffic.
  - Four accumulation chains share each PSUM bank; only the chain writing
    first uses start=True (clears the whole bank's has_written bits), the
    others overwrite-where-unset.
  - All deferred work (V projection chunks, AV+finalize, later Q/K
    projections) sits in queues annotated with the earliest "slot" (ACT
    count) at which its input DMA will have landed, so a not-yet-ready
    instruction never enters the PE FIFO ahead of the scores matmuls that
    feed ScalarE. AV closures additionally gate on their V chunk being
    emitted; finalize closures ride the same queue so o_ps frees in order.
  - inputs stream as column-sliced DMAs in consumption order (the DMA
    engine pool is serial in the cost model): wq, wk, xk0, xq0, xk1, xv0,
    xk2, xv1, xk3, xq1, xv2, xv3, xq2, xq3.

PSUM (8 banks): scores 2bufs x [128,2,512] f32 = 4, O accum 2 (2qc x 2h x 66
x 2 banks), projection staging 2.
"""
import numpy as np
import ml_dtypes
from contextlib import ExitStack

import concourse.tile as tile
import concourse.mybir as mybir
from concourse import bacc
from concourse.bass_utils import run_bass_kernel_spmd

P = 128
DH = 64
F16 = mybir.dt.float16
F32 = mybir.dt.float32

AV_START = 2          # earliest kc for AV draining (first block)
AV_RATE = 6           # max AV/finalize closures drained per kc
V_RATE = 3            # max V-projection closures drained per kc
# earliest global slot for V chunk group c//4 (when xv quarter c//4 landed)
V_SLOTS = (13, 16, 19, 22)
PQ_RATE = 3           # max projection closures drained per kc


def build_attention(S=2048, D=1024, HPC=8, loop_n=1, pops=PQ_RATE):
    """Build the per-core SPMD program. HPC = heads per core (even).

    loop_n > 1 wraps the whole body in a hardware loop (for timing)."""
    DC = D // P        # D chunks of 128
    KC = S // P        # k chunks of 128
    NQ = S // 512      # q blocks of 512
    HP = HPC // 2      # head pairs
    CW = HPC * DH      # core output width
    ACT_SCALE = 1.0 / float(np.sqrt(DH))

    nc = bacc.Bacc("TRN2")
    xq = nc.dram_tensor("xq", [P, DC, S], F16, kind="ExternalInput")
    xk = nc.dram_tensor("xk", [P, DC, S], F16, kind="ExternalInput")
    xv = nc.dram_tensor("xv", [P, DC, S], F16, kind="ExternalInput")
    wq = nc.dram_tensor("wq", [P, DC, CW], F16, kind="ExternalInput")
    wk = nc.dram_tensor("wk", [P, DC, CW], F16, kind="ExternalInput")
    wv = nc.dram_tensor("wv", [P, DC, CW], F16, kind="ExternalInput")
    out = nc.dram_tensor("out", [HPC, P, NQ, 4, DH], F32, kind="ExternalOutput")

    with tile.TileContext(nc) as tc, ExitStack() as ctx:
        xpool = ctx.enter_context(tc.tile_pool(name="x", bufs=1))
        wpool = ctx.enter_context(tc.tile_pool(name="w", bufs=1))
        vpool = ctx.enter_context(tc.tile_pool(name="v", bufs=1))
        qkpool = ctx.enter_context(tc.tile_pool(name="qk", bufs=3))
        ppool = ctx.enter_context(tc.tile_pool(name="p", bufs=20))
        rpool = ctx.enter_context(tc.tile_pool(name="r", bufs=4))
        opool = ctx.enter_context(tc.tile_pool(name="ob", bufs=2))
        otpool = ctx.enter_context(tc.tile_pool(name="ot", bufs=2))
        ps_s = ctx.enter_context(tc.tile_pool(name="ps_s", bufs=2, space="PSUM"))
        ps_o = ctx.enter_context(tc.tile_pool(name="ps_o", bufs=1, space="PSUM"))
        ps_m = ctx.enter_context(tc.tile_pool(name="ps_m", bufs=2, space="PSUM"))

        xs, ws = {}, {}
        vta = None
        slot = [0]           # global ACT counter
        vta_done = [False] * KC

        def emit_loads():
            nonlocal vta
            for name in ("q", "k", "v"):
                ws[name] = wpool.tile([P, DC, CW], F16, tag="w" + name,
                                      name="w" + name)
                xs[name] = xpool.tile([P, DC, S], F16, tag="x" + name,
                                      name="x" + name)

            def ld(t, dram, c0, c1):
                nc.sync.dma_start(t[:, :, c0:c1], dram[:, :, c0:c1])

            # DMA order = consumption order (DMA engine pool is serial).
            # hp0's weight columns first; the rest of W after the k/v bulk.
            nc.sync.dma_start(ws["q"][:, :, 0:P], wq[:, :, 0:P])
            nc.sync.dma_start(ws["k"][:, :, 0:P], wk[:, :, 0:P])
            ld(xs["q"], xq, 0, 512)
            ld(xs["k"], xk, 0, 512)
            ld(xs["k"], xk, 512, 1024)
            ld(xs["k"], xk, 1024, 1536)
            nc.sync.dma_start(ws["v"][:], wv[:])
            ld(xs["k"], xk, 1536, 2048)
            ld(xs["v"], xv, 0, 512)
            ld(xs["q"], xq, 512, 1024)
            ld(xs["v"], xv, 512, 1024)
            ld(xs["v"], xv, 1024, 1536)
            ld(xs["v"], xv, 1536, 2048)
            nc.sync.dma_start(ws["q"][:, :, P:CW], wq[:, :, P:CW])
            nc.sync.dma_start(ws["k"][:, :, P:CW], wk[:, :, P:CW])
            ld(xs["q"], xq, 1024, 1536)
            ld(xs["q"], xq, 1536, 2048)
            # V-augmented rhs: [kpos, kc, ch, 66] = V | 1.0 | 0 pad
            vta = vpool.tile([P, KC, HPC, 66], F16, tag="V", name="vta")
            nc.vector.memset(vta[:, :, :, 64], 1.0)
            nc.vector.memset(vta[:, :, :, 65], 0.0)

        def v_closures(kc, min_slot):
            pstate = {}

            def mk(dc):
                def f():
                    if dc == 0:
                        pstate["pv"] = ps_m.tile([P, 512], F32,
                                                 tag="proj", name="pv")
                    nc.tensor.matmul(
                        pstate["pv"][:, :CW],
                        xs["v"][:, dc, kc * P : (kc + 1) * P],
                        ws["v"][:, dc, :],
                        start=(dc == 0),
                        stop=(dc == DC - 1),
                    )
                    if dc == DC - 1:
                        nc.vector.tensor_copy(
                            vta[:, kc, :, 0:DH],
                            pstate["pv"][:, :CW].rearrange(
                                "p (h d) -> p h d", d=DH),
                        )
                        vta_done[kc] = True
                return f

            return [(min_slot, mk(d)) for d in range(DC)]

        def new_qk(which):
            return qkpool.tile([P, S], F16, tag=which, name=which + "t")

        def proj_qk_chunk(t, which, hp, qb):
            pp = ps_m.tile([P, 512], F32, tag="proj", name="pp")
            for dc in range(DC):
                nc.tensor.matmul(
                    pp[:],
                    ws[which][:, dc, hp * P : (hp + 1) * P],
                    xs[which][:, dc, qb * 512 : (qb + 1) * 512],
                    start=(dc == 0),
                    stop=(dc == DC - 1),
                )
            nc.vector.tensor_copy(t[:, qb * 512 : (qb + 1) * 512], pp[:])

        def chunk_closures(t, which, hp, qb, min_slot, c0=0, c1=512):
            """(min_slot, closure) items: one per matmul; last also
            evacuates. c0:c1 select columns within the 512-wide chunk."""
            pstate = {}
            w = c1 - c0

            def mk(dc):
                def f():
                    if dc == 0:
                        pstate["pp"] = ps_m.tile([P, 512], F32,
                                                 tag="proj", name="pp")
                    nc.tensor.matmul(
                        pstate["pp"][:, 0:w],
                        ws[which][:, dc, hp * P : (hp + 1) * P],
                        xs[which][:, dc, qb * 512 + c0 : qb * 512 + c1],
                        start=(dc == 0),
                        stop=(dc == DC - 1),
                    )
                    if dc == DC - 1:
                        nc.vector.tensor_copy(
                            t[:, qb * 512 + c0 : qb * 512 + c1],
                            pstate["pp"][:, 0:w])
                return f

            return [(min_slot, mk(d)) for d in range(DC)]

        def drain(q, budget, gate=None):
            while budget and q:
                head = q[0]
                if head[0] is not None and head[0] > slot[0]:
                    break
                if gate is not None and not gate(head):
                    break
                q.pop(0)[1]()
                budget -= 1

        def attn_block(hp, qb, qt, kt, proj_q, v_q, av_q, kt_done, q0_q, first_hp=False, last=False):
            # o banks: [128(q), 2(qc half), 2(head), 66]; qc 0,1 -> bank A,
            # qc 2,3 -> bank B
            o_ps = [ps_o.tile([P, 2, 2, 66], F32, tag=f"O{i}", name=f"o{i}")
                    for i in (0, 1)]

            def emit_scores(kc):
                s = ps_s.tile([P, 2, 512], F32, tag="S", name="s")
                for h in (0, 1):
                    nc.tensor.matmul(
                        s[:, h, :],
                        kt[h * DH : (h + 1) * DH, kc * P : (kc + 1) * P],
                        qt[h * DH : (h + 1) * DH, qb * 512 : (qb + 1) * 512],
                        start=True,
                        stop=True,
                    )
                return s

            def mk_av(kc, pt):
                def f():
                    for qc in range(4):
                        for h in (0, 1):
                            nc.tensor.matmul(
                                o_ps[qc // 2][:, qc % 2, h, :],
                                pt[:, h, qc * P : (qc + 1) * P],
                                vta[:, kc, hp * 2 + h, :],
                                start=(kc == 0 and qc % 2 == 0 and h == 0),
                                stop=(kc == KC - 1),
                                skip_group_check=(qc + h > 0),
                            )
                return f

            def finalize():
                ot = otpool.tile([P, 4, 2, DH], F32, tag="ot", name="ot")
                for i in (0, 1):
                    osb = opool.tile([P, 2, 2, 66], F32, tag="osb", name="osb")
                    nc.vector.tensor_copy(osb[:], o_ps[i][:])
                    for j in (0, 1):
                        for h in (0, 1):
                            rt = rpool.tile([P, 1], F32, tag="rt", name="rt")
                            nc.vector.reciprocal(rt[:], osb[:, j, h, 64:65])
                            nc.vector.tensor_scalar_mul(
                                ot[:, 2 * i + j, h, :], osb[:, j, h, 0:DH],
                                rt[:, 0:1])
                for h in (0, 1):
                    ch = hp * 2 + h
                    nc.sync.dma_start(out[ch, :, qb, :, :], ot[:, :, h, :])

            def gate_av(head):
                kc = head[2]
                return kc is None or vta_done[kc]

            s_cur = emit_scores(0)
            for kc in range(KC):
                pt = ppool.tile([P, 2, 512], F16, tag="p", name="pt")
                nc.scalar.activation(
                    pt[:], s_cur[:],
                    mybir.ActivationFunctionType.Exp,
                    scale=ACT_SCALE)
                slot[0] += 1
                if kc + 1 < KC:
                    if first_hp and qb == 0:
                        # force-drain deferred kt work until the columns the
                        # next scores matmul reads have been projected
                        while kt_done[0] < (kc + 2) * P and proj_q:
                            assert proj_q[0][2] == -1
                            proj_q.pop(0)[1]()
                    s_cur = emit_scores(kc + 1)
                drain(v_q, V_RATE)
                av_q.append((None, mk_av(kc, pt), kc))
                drain(av_q, len(av_q) if last else AV_RATE, gate=gate_av)
                drain(q0_q, 2)
                drain(proj_q, pops)
            av_q.append((None, finalize, None))

        def emit_body():
            emit_loads()
            qt = new_qk("q")
            kt = new_qk("k")
            # warm the PE p-state during the input-DMA wait: dummy
            # matmuls over the zeroed vta keep the ramp model at full speed
            # for the first real projections
            oc = vta[:, 0, :, 64:66]   # [P, HPC, 2] initialized slice
            for i in range(50):
                wp = ps_m.tile([P, 512], F32, tag="proj", name="wp")
                nc.tensor.matmul(
                    wp[0:1, 0:256],
                    oc[:, 0, 0:1],
                    oc.to_broadcast((P, HPC, 2, 16)),
                    start=True, stop=True)
            # prologue: just enough projection for the first scores:
            # Q chunk 0 (xq0 lands first), then K chunk 0 cols 0:256.
            proj_qk_chunk(qt, "q", 0, 0)
            for _s, f in chunk_closures(kt, "k", 0, 0, 0, 0, 256):
                f()
            # deferred, force-drained ahead of the scores that read them
            # (tag -1 entries carry kt columns; kt_cols tracks progress)
            proj_q = []
            kt_done = [256]

            def mark(cols):
                def g():
                    kt_done[0] = cols
                return g

            proj_q += [(0, f, -1) for _s, f in
                       chunk_closures(kt, "k", 0, 0, 0, 256, 512)]
            proj_q.append((0, mark(512), -1))
            proj_q += [(0, f, -1) for _s, f in
                       chunk_closures(kt, "k", 0, 1, 0)]
            proj_q.append((0, mark(1024), -1))
            proj_q += [(4, f, -1) for _s, f in
                       chunk_closures(kt, "k", 0, 2, 4)]
            proj_q.append((4, mark(1536), -1))
            proj_q += [(8, f, -1) for _s, f in
                       chunk_closures(kt, "k", 0, 3, 8)]
            proj_q.append((8, mark(2048), -1))
            q0_q = []
            for qb, ms in ((1, 15), (2, 29), (3, 32)):
                q0_q += [(s0, f, qb) for s0, f in
                         chunk_closures(qt, "q", 0, qb, ms)]
            v_q = []
            for kc in range(KC):
                v_q += v_closures(kc, V_SLOTS[kc // 4] + 2 * (kc % 4))
            av_q = []

            # prefetch queues for hp 1..3, tagged with their hp so the
            # boundary flush can force-complete exactly what's needed
            qts = {0: (qt, kt)}
            for hpn in range(1, HP):
                base = (27, 50, 104, 168)[hpn]
                qts[hpn] = (new_qk("q"), new_qk("k"))
                for qb in range(NQ):
                    proj_q += [(max(s0, base), f, hpn) for s0, f in
                               chunk_closures(qts[hpn][1], "k", hpn, qb, 0)]
                for qb in range(NQ):
                    proj_q += [(max(s0, base, 26), f, hpn) for s0, f in
                               chunk_closures(qts[hpn][0], "q", hpn, qb, 0)]
            for hp in range(HP):
                qt, kt = qts[hp]
                for qb in range(NQ):
                    if hp == 0:
                        # hp0's qt chunk qb must be fully projected before
                        # this block's scores read it
                        while q0_q and q0_q[0][2] <= qb:
                            q0_q.pop(0)[1]()
                    attn_block(hp, qb, qt, kt, proj_q, v_q, av_q, kt_done,
                               q0_q if hp == 0 else [],
                               first_hp=(hp == 0),
                               last=(hp == HP - 1 and qb == NQ - 1))
                # next head pair's projections must be fully emitted before
                # its attention reads them
                if hp + 1 < HP:
                    while proj_q and proj_q[0][2] <= hp + 1:
                        proj_q.pop(0)[1]()
            while v_q:
                v_q.pop(0)[1]()
            while av_q:
                av_q.pop(0)[1]()

        if loop_n > 1:
            with tc.For_i(0, loop_n, 1):
                emit_body()
        else:
            emit_body()

    nc.compile()
    return nc


_NC_CACHE = {}


def _get_nc(S, D, HPC):
    key = (S, D, HPC)
    if key not in _NC_CACHE:
        _NC_CACHE[key] = build_attention(S, D, HPC)
    return _NC_CACHE[key]


def _prep_batch_x(q_seq, k_seq, v_seq, b, D):
    """Per-batch fp16 x^T shards (shared by the 2 head-group cores)."""
    DC = D // P

    def xt(x):  # [S, D] -> [P, DC, S]
        return np.ascontiguousarray(
            x.T.reshape(DC, P, -1).transpose(1, 0, 2)).astype(np.float16)

    return {"xq": xt(q_seq[b]), "xk": xt(k_seq[b]), "xv": xt(v_seq[b])}


def _prep_w(WQ, WK, WV, hg, HPC, D):
    """Per-head-group fp16 weight shards."""
    DC = D // P
    CW = HPC * DH

    def wslice(w):  # [D, out] -> [P, DC, CW]
        return np.ascontiguousarray(
            w[:, hg * CW : (hg + 1) * CW]
            .reshape(DC, P, CW).transpose(1, 0, 2)).astype(np.float16)

    return {"wq": wslice(WQ), "wk": wslice(WK), "wv": wslice(WV)}


def _prep_core_inputs(q_seq, k_seq, v_seq, WQ, WK, WV, b, hg, HPC, D):
    """Host-side shard prep for core (batch b, head group hg)."""
    m = _prep_batch_x(q_seq, k_seq, v_seq, b, D)
    m.update(_prep_w(WQ, WK, WV, hg, HPC, D))
    return m


def kernel(q_seq, k_seq, v_seq, WQ, WK, WV, _trace=False):
    q_seq = np.asarray(q_seq, dtype=np.float32)
    k_seq = np.asarray(k_seq, dtype=np.float32)
    v_seq = np.asarray(v_seq, dtype=np.float32)
    WQ = np.asarray(WQ, dtype=np.float32)
    WK = np.asarray(WK, dtype=np.float32)
    WV = np.asarray(WV, dtype=np.float32)

    B, S, D = q_seq.shape
    NB_HEAD = WQ.shape[1] // DH
    n_cores = 8
    groups_per_batch = n_cores // B          # 2 head groups
    HPC = NB_HEAD // groups_per_batch        # 8 heads per core
    CW = HPC * DH

    nc = _get_nc(S, D, HPC)

    xmaps = {b: _prep_batch_x(q_seq, k_seq, v_seq, b, D) for b in range(B)}
    wmaps = {hg: _prep_w(WQ, WK, WV, hg, HPC, D) for hg in range(groups_per_batch)}
    in_maps = []
    for core in range(n_cores):
        b, hg = core // groups_per_batch, core % groups_per_batch
        in_maps.append({**xmaps[b], **wmaps[hg]})

    res = run_bass_kernel_spmd(
        nc, in_maps, core_ids=list(range(n_cores)), trace=_trace,
        **({"trace_cores": [0], } if _trace else {}),
    )
    if _trace:
        print(f"HW exec time: {res.exec_time_ns} ns")
        if res.instructions_and_trace:
            print("trace:", res.instructions_and_trace[1])

    out = np.empty((B, S, NB_HEAD * DH), dtype=np.float32)
    for core in range(n_cores):
        b, hg = core // groups_per_batch, core % groups_per_batch
        # device output: [HPC, P, NQ, 4, DH]; q = qb*512 + qc*128 + p
        ot = res.results[core]["out"]
        ot = ot.transpose(2, 3, 1, 0, 4).reshape(S, CW)
        out[b, :, hg * CW : (hg + 1) * CW] = ot
    return out


# revision 35
# speedup vs baseline: 2.2517x; 1.0043x over previous
"""Multi-head attention Bass kernel for Trainium2, SPMD over 8 NeuronCores.

Problem: B=4, S=2048, D=1024, 16 heads x 64. Sharding: core = (batch b, head-group hg)
with b in 0..3, hg in 0..1 -> each core computes 8 heads of one batch.

Design (cost-model driven, fp16 end-to-end):
  - ScalarE's exp is the hard floor: 256 activations of [128(k), 2(head),
    512(q)] PSUM fp32 -> p fp16, ~1.04us each = ~266us. Everything else is
    arranged to hide under it.
  - scores (PE): per (kc, head) one fp16 matmul K=64 -> s[k, q] in PSUM.
  - AV is Q-MAJOR: O[q, dh] accumulates with M=128 q-positions on PSUM
    partitions and only N=66 columns (64 dh + a ones column that picks up
    the softmax denominator for free + 1 pad for 8B alignment). lhsT is
    the p tile (stationary), rhs is V-augmented [k, 66]. Cost: 66 cycles
    per (kc, head, q-128-chunk) -> ~58us total, half of the k-major form,
    and the denominator lands per-partition-aligned with q so normalize is
    reciprocal + tensor_scalar_mul per chunk - no cross-partition traffic.
  - Four accumulation chains share each PSUM bank; only the chain writing
    first uses start=True (clears the whole bank's has_written bits), the
    others overwrite-where-unset.
  - All deferred work (V projection chunks, AV+finalize, later Q/K
    projections) sits in queues annotated with the earliest "slot" (ACT
    count) at which its input DMA will have landed, so a not-yet-ready
    instruction never enters the PE FIFO ahead of the scores matmuls that
    feed ScalarE. AV closures additionally gate on their V chunk being
    emitted; finalize closures ride the same queue so o_ps frees in order.
  - inputs stream as column-sliced DMAs in consumption order (the DMA
    engine pool is serial in the cost model): wq, wk, xk0, xq0, xk1, xv0,
    xk2, xv1, xk3, xq1, xv2, xv3, xq2, xq3.

PSUM (8 banks): scores 2bufs x [128,2,512] f32 = 4, O accum 2 (2qc x 2h x 66
x 2 banks), projection staging 2.
"""
import numpy as np
import ml_dtypes
from contextlib import ExitStack

import concourse.tile as tile
import concourse.mybir as mybir
from concourse import bacc
from concourse.bass_utils import run_bass_kernel_spmd

P = 128
DH = 64
F16 = mybir.dt.float16
F32 = mybir.dt.float32

AV_START = 2          # earliest kc for AV draining (first block)
AV_RATE = 6           # max AV/finalize closures drained per kc
V_RATE = 3            # max V-projection closures drained per kc
# earliest global slot for V chunk group c//4 (when xv quarter c//4 landed)
V_SLOTS = (13, 16, 19, 22)
PQ_RATE = 3           # max projection closures drained per kc


def build_attention(S=2048, D=1024, HPC=8, loop_n=1, pops=PQ_RATE):
    """Build the per-core SPMD program. HPC = heads per core (even).

    loop_n > 1 wraps the whole body in a hardware loop (for timing)."""
    DC = D // P        # D chunks of 128
    KC = S // P        # k chunks of 128
    NQ = S // 512      # q blocks of 512
    HP = HPC // 2      # head pairs
    CW = HPC * DH      # core output width
    ACT_SCALE = 1.0 / float(np.sqrt(DH))

    nc = bacc.Bacc("TRN2")
    xq = nc.dram_tensor("xq", [P, DC, S], F16, kind="ExternalInput")
    xk = nc.dram_tensor("xk", [P, DC, S], F16, kind="ExternalInput")
    xv = nc.dram_tensor("xv", [P, DC, S], F16, kind="ExternalInput")
    wq = nc.dram_tensor("wq", [P, DC, CW], F16, kind="ExternalInput")
    wk = nc.dram_tensor("wk", [P, DC, CW], F16, kind="ExternalInput")
    wv = nc.dram_tensor("wv", [P, DC, CW], F16, kind="ExternalInput")
    out = nc.dram_tensor("out", [HPC, P, NQ, 4, DH], F32, kind="ExternalOutput")

    with tile.TileContext(nc) as tc, ExitStack() as ctx:
        xpool = ctx.enter_context(tc.tile_pool(name="x", bufs=1))
        wpool = ctx.enter_context(tc.tile_pool(name="w", bufs=1))
        vpool = ctx.enter_context(tc.tile_pool(name="v", bufs=1))
        qkpool = ctx.enter_context(tc.tile_pool(name="qk", bufs=3))
        ppool = ctx.enter_context(tc.tile_pool(name="p", bufs=20))
        rpool = ctx.enter_context(tc.tile_pool(name="r", bufs=4))
        opool = ctx.enter_context(tc.tile_pool(name="ob", bufs=2))
        otpool = ctx.enter_context(tc.tile_pool(name="ot", bufs=2))
        ps_s = ctx.enter_context(tc.tile_pool(name="ps_s", bufs=2, space="PSUM"))
        ps_o = ctx.enter_context(tc.tile_pool(name="ps_o", bufs=1, space="PSUM"))
        ps_m = ctx.enter_context(tc.tile_pool(name="ps_m", bufs=2, space="PSUM"))

        xs, ws = {}, {}
        vta = None
        slot = [0]           # global ACT counter
        vta_done = [False] * KC

        def emit_loads():
            nonlocal vta
            for name in ("q", "k", "v"):
                ws[name] = wpool.tile([P, DC, CW], F16, tag="w" + name,
                                      name="w" + name)
                xs[name] = xpool.tile([P, DC, S], F16, tag="x" + name,
                                      name="x" + name)

            def ld(t, dram, c0, c1):
                nc.sync.dma_start(t[:, :, c0:c1], dram[:, :, c0:c1])

            # DMA order = consumption order (DMA engine pool is serial).
            # hp0's weight columns first; the rest of W after the k/v bulk.
            nc.sync.dma_start(ws["q"][:, :, 0:P], wq[:, :, 0:P])
            nc.sync.dma_start(ws["k"][:, :, 0:P], wk[:, :, 0:P])
            ld(xs["q"], xq, 0, 512)
            ld(xs["k"], xk, 0, 512)
            ld(xs["k"], xk, 512, 1024)
            ld(xs["k"], xk, 1024, 1536)
            nc.sync.dma_start(ws["v"][:], wv[:])
            ld(xs["k"], xk, 1536, 2048)
            ld(xs["v"], xv, 0, 512)
            ld(xs["q"], xq, 512, 1024)
            ld(xs["v"], xv, 512, 1024)
            ld(xs["v"], xv, 1024, 1536)
            ld(xs["v"], xv, 1536, 2048)
            nc.sync.dma_start(ws["q"][:, :, P:CW], wq[:, :, P:CW])
            nc.sync.dma_start(ws["k"][:, :, P:CW], wk[:, :, P:CW])
            ld(xs["q"], xq, 1024, 1536)
            ld(xs["q"], xq, 1536, 2048)
            # V-augmented rhs: [kpos, kc, ch, 66] = V | 1.0 | 0 pad
            vta = vpool.tile([P, KC, HPC, 66], F16, tag="V", name="vta")
            nc.vector.memset(vta[:, :, :, 64], 1.0)
            nc.vector.memset(vta[:, :, :, 65], 0.0)

        def v_closures(kc, min_slot):
            pstate = {}

            def mk(dc):
                def f():
                    if dc == 0:
                        pstate["pv"] = ps_m.tile([P, 512], F32,
                                                 tag="proj", name="pv")
                    nc.tensor.matmul(
                        pstate["pv"][:, :CW],
                        xs["v"][:, dc, kc * P : (kc + 1) * P],
                        ws["v"][:, dc, :],
                        start=(dc == 0),
                        stop=(dc == DC - 1),
                    )
                    if dc == DC - 1:
                        nc.vector.tensor_copy(
                            vta[:, kc, :, 0:DH],
                            pstate["pv"][:, :CW].rearrange(
                                "p (h d) -> p h d", d=DH),
                        )
                        vta_done[kc] = True
                return f

            return [(min_slot, mk(d)) for d in range(DC)]

        def new_qk(which):
            return qkpool.tile([P, S], F16, tag=which, name=which + "t")

        def proj_qk_chunk(t, which, hp, qb):
            pp = ps_m.tile([P, 512], F32, tag="proj", name="pp")
            for dc in range(DC):
                nc.tensor.matmul(
                    pp[:],
                    ws[which][:, dc, hp * P : (hp + 1) * P],
                    xs[which][:, dc, qb * 512 : (qb + 1) * 512],
                    start=(dc == 0),
                    stop=(dc == DC - 1),
                )
            nc.vector.tensor_copy(t[:, qb * 512 : (qb + 1) * 512], pp[:])

        def chunk_closures(t, which, hp, qb, min_slot, c0=0, c1=512):
            """(min_slot, closure) items: one per matmul; last also
            evacuates. c0:c1 select columns within the 512-wide chunk."""
            pstate = {}
            w = c1 - c0

            def mk(dc):
                def f():
                    if dc == 0:
                        pstate["pp"] = ps_m.tile([P, 512], F32,
                                                 tag="proj", name="pp")
                    nc.tensor.matmul(
                        pstate["pp"][:, 0:w],
                        ws[which][:, dc, hp * P : (hp + 1) * P],
                        xs[which][:, dc, qb * 512 + c0 : qb * 512 + c1],
                        start=(dc == 0),
                        stop=(dc == DC - 1),
                    )
                    if dc == DC - 1:
                        nc.vector.tensor_copy(
                            t[:, qb * 512 + c0 : qb * 512 + c1],
                            pstate["pp"][:, 0:w])
                return f

            return [(min_slot, mk(d)) for d in range(DC)]

        def drain(q, budget, gate=None):
            while budget and q:
                head = q[0]
                if head[0] is not None and head[0] > slot[0]:
                    break
                if gate is not None and not gate(head):
                    break
                q.pop(0)[1]()
                budget -= 1

        def attn_block(hp, qb, qt, kt, proj_q, v_q, av_q, kt_done, q0_q, first_hp=False, last=False):
            # o banks: [128(q), 2(qc half), 2(head), 66]; qc 0,1 -> bank A,
            # qc 2,3 -> bank B
            o_ps = [ps_o.tile([P, 2, 2, 66], F32, tag=f"O{i}", name=f"o{i}")
                    for i in (0, 1)]

            def emit_scores(kc):
                s = ps_s.tile([P, 2, 512], F32, tag="S", name="s")
                for h in (0, 1):
                    nc.tensor.matmul(
                        s[:, h, :],
                        kt[h * DH : (h + 1) * DH, kc * P : (kc + 1) * P],
                        qt[h * DH : (h + 1) * DH, qb * 512 : (qb + 1) * 512],
                        start=True,
                        stop=True,
                    )
                return s

            def mk_av(kc, pt):
                def f():
                    for qc in range(4):
                        for h in (0, 1):
                            nc.tensor.matmul(
                                o_ps[qc // 2][:, qc % 2, h, :],
                                pt[:, h, qc * P : (qc + 1) * P],
                                vta[:, kc, hp * 2 + h, :],
                                start=(kc == 0 and qc % 2 == 0 and h == 0),
                                stop=(kc == KC - 1),
                                skip_group_check=(qc + h > 0),
                            )
                return f

            def finalize():
                ot = otpool.tile([P, 4, 2, DH], F32, tag="ot", name="ot")
                for i in (0, 1):
                    osb = opool.tile([P, 2, 2, 66], F32, tag="osb", name="osb")
                    nc.vector.tensor_copy(osb[:], o_ps[i][:])
                    for j in (0, 1):
                        for h in (0, 1):
                            rt = rpool.tile([P, 1], F32, tag="rt", name="rt")
                            nc.vector.reciprocal(rt[:], osb[:, j, h, 64:65])
                            nc.vector.tensor_scalar_mul(
                                ot[:, 2 * i + j, h, :], osb[:, j, h, 0:DH],
                                rt[:, 0:1])
                for h in (0, 1):
                    ch = hp * 2 + h
                    nc.sync.dma_start(out[ch, :, qb, :, :], ot[:, :, h, :])

            def gate_av(head):
                kc = head[2]
                return kc is None or vta_done[kc]

            s_cur = emit_scores(0)
            for kc in range(KC):
                pt = ppool.tile([P, 2, 512], F16, tag="p", name="pt")
                nc.scalar.activation(
                    pt[:], s_cur[:],
                    mybir.ActivationFunctionType.Exp,
                    scale=ACT_SCALE)
                slot[0] += 1
                if kc + 1 < KC:
                    if first_hp and qb == 0:
                        # force-drain deferred kt work until the columns the
                        # next scores matmul reads have been projected
                        while kt_done[0] < (kc + 2) * P and proj_q:
                            assert proj_q[0][2] == -1
                            proj_q.pop(0)[1]()
                    s_cur = emit_scores(kc + 1)
                drain(v_q, V_RATE)
                av_q.append((None, mk_av(kc, pt), kc))
                drain(av_q, len(av_q) if last else AV_RATE, gate=gate_av)
                drain(q0_q, 2)
                drain(proj_q, pops)
            av_q.append((None, finalize, None))

        def emit_body():
            emit_loads()
            qt = new_qk("q")
            kt = new_qk("k")
            # warm the PE p-state during the input-DMA wait: dummy
            # matmuls over the zeroed vta keep the ramp model at full speed
            # for the first real projections
            oc = vta[:, 0, :, 64:66]   # [P, HPC, 2] initialized slice
            for i in range(50):
                wp = ps_m.tile([P, 512], F32, tag="proj", name="wp")
                nc.tensor.matmul(
                    wp[0:1, 0:256],
                    oc[:, 0, 0:1],
                    oc.to_broadcast((P, HPC, 2, 16)),
                    start=True, stop=True)
            # prologue: just enough projection for the first scores:
            # Q chunk 0 (xq0 lands first), then K chunk 0 cols 0:256.
            proj_qk_chunk(qt, "q", 0, 0)
            for _s, f in chunk_closures(kt, "k", 0, 0, 0, 0, 256):
                f()
            # deferred, force-drained ahead of the scores that read them
            # (tag -1 entries carry kt columns; kt_cols tracks progress)
            proj_q = []
            kt_done = [256]

            def mark(cols):
                def g():
                    kt_done[0] = cols
                return g

            proj_q += [(0, f, -1) for _s, f in
                       chunk_closures(kt, "k", 0, 0, 0, 256, 512)]
            proj_q.append((0, mark(512), -1))
            proj_q += [(0, f, -1) for _s, f in
                       chunk_closures(kt, "k", 0, 1, 0)]
            proj_q.append((0, mark(1024), -1))
            proj_q += [(4, f, -1) for _s, f in
                       chunk_closures(kt, "k", 0, 2, 4)]
            proj_q.append((4, mark(1536), -1))
            proj_q += [(8, f, -1) for _s, f in
                       chunk_closures(kt, "k", 0, 3, 8)]
            proj_q.append((8, mark(2048), -1))
            q0_q = []
            for qb, ms in ((1, 15), (2, 29), (3, 32)):
                q0_q += [(s0, f, qb) for s0, f in
                         chunk_closures(qt, "q", 0, qb, ms)]
            v_q = []
            for kc in range(KC):
                v_q += v_closures(kc, V_SLOTS[kc // 4] + 2 * (kc % 4))
            av_q = []

            # prefetch queues for hp 1..3, tagged with their hp so the
            # boundary flush can force-complete exactly what's needed
            qts = {0: (qt, kt)}
            for hpn in range(1, HP):
                base = (27, 56, 104, 170)[hpn]
                qts[hpn] = (new_qk("q"), new_qk("k"))
                for qb in range(NQ):
                    proj_q += [(max(s0, base), f, hpn) for s0, f in
                               chunk_closures(qts[hpn][1], "k", hpn, qb, 0)]
                for qb in range(NQ):
                    proj_q += [(max(s0, base, 26), f, hpn) for s0, f in
                               chunk_closures(qts[hpn][0], "q", hpn, qb, 0)]
            for hp in range(HP):
                qt, kt = qts[hp]
                for qb in range(NQ):
                    if hp == 0:
                        # hp0's qt chunk qb must be fully projected before
                        # this block's scores read it
                        while q0_q and q0_q[0][2] <= qb:
                            q0_q.pop(0)[1]()
                    attn_block(hp, qb, qt, kt, proj_q, v_q, av_q, kt_done,
                               q0_q if hp == 0 else [],
                               first_hp=(hp == 0),
                               last=(hp == HP - 1 and qb == NQ - 1))
                # next head pair's projections must be fully emitted before
                # its attention reads them
                if hp + 1 < HP:
                    while proj_q and proj_q[0][2] <= hp + 1:
                        proj_q.pop(0)[1]()
            while v_q:
                v_q.pop(0)[1]()
            while av_q:
                av_q.pop(0)[1]()

        if loop_n > 1:
            with tc.For_i(0, loop_n, 1):
                emit_body()
        else:
            emit_body()

    nc.compile()
    return nc


_NC_CACHE = {}


def _get_nc(S, D, HPC):
    key = (S, D, HPC)
    if key not in _NC_CACHE:
        _NC_CACHE[key] = build_attention(S, D, HPC)
    return _NC_CACHE[key]


def _prep_batch_x(q_seq, k_seq, v_seq, b, D):
    """Per-batch fp16 x^T shards (shared by the 2 head-group cores)."""
    DC = D // P

    def xt(x):  # [S, D] -> [P, DC, S]
        return np.ascontiguousarray(
            x.T.reshape(DC, P, -1).transpose(1, 0, 2)).astype(np.float16)

    return {"xq": xt(q_seq[b]), "xk": xt(k_seq[b]), "xv": xt(v_seq[b])}


def _prep_w(WQ, WK, WV, hg, HPC, D):
    """Per-head-group fp16 weight shards."""
    DC = D // P
    CW = HPC * DH

    def wslice(w):  # [D, out] -> [P, DC, CW]
        return np.ascontiguousarray(
            w[:, hg * CW : (hg + 1) * CW]
            .reshape(DC, P, CW).transpose(1, 0, 2)).astype(np.float16)

    return {"wq": wslice(WQ), "wk": wslice(WK), "wv": wslice(WV)}


def _prep_core_inputs(q_seq, k_seq, v_seq, WQ, WK, WV, b, hg, HPC, D):
    """Host-side shard prep for core (batch b, head group hg)."""
    m = _prep_batch_x(q_seq, k_seq, v_seq, b, D)
    m.update(_prep_w(WQ, WK, WV, hg, HPC, D))
    return m


def kernel(q_seq, k_seq, v_seq, WQ, WK, WV, _trace=False):
    q_seq = np.asarray(q_seq, dtype=np.float32)
    k_seq = np.asarray(k_seq, dtype=np.float32)
    v_seq = np.asarray(v_seq, dtype=np.float32)
    WQ = np.asarray(WQ, dtype=np.float32)
    WK = np.asarray(WK, dtype=np.float32)
    WV = np.asarray(WV, dtype=np.float32)

    B, S, D = q_seq.shape
    NB_HEAD = WQ.shape[1] // DH
    n_cores = 8
    groups_per_batch = n_cores // B          # 2 head groups
    HPC = NB_HEAD // groups_per_batch        # 8 heads per core
    CW = HPC * DH

    nc = _get_nc(S, D, HPC)

    xmaps = {b: _prep_batch_x(q_seq, k_seq, v_seq, b, D) for b in range(B)}
    wmaps = {hg: _prep_w(WQ, WK, WV, hg, HPC, D) for hg in range(groups_per_batch)}
    in_maps = []
    for core in range(n_cores):
        b, hg = core // groups_per_batch, core % groups_per_batch
        in_maps.append({**xmaps[b], **wmaps[hg]})

    res = run_bass_kernel_spmd(
        nc, in_maps, core_ids=list(range(n_cores)), trace=_trace,
        **({"trace_cores": [0], } if _trace else {}),
    )
    if _trace:
        print(f"HW exec time: {res.exec_time_ns} ns")
        if res.instructions_and_trace:
            print("trace:", res.instructions_and_trace[1])

    out = np.empty((B, S, NB_HEAD * DH), dtype=np.float32)
    for core in range(n_cores):
        b, hg = core // groups_per_batch, core % groups_per_batch
        # device output: [HPC, P, NQ, 4, DH]; q = qb*512 + qc*128 + p
        ot = res.results[core]["out"]
        ot = ot.transpose(2, 3, 1, 0, 4).reshape(S, CW)
        out[b, :, hg * CW : (hg + 1) * CW] = ot
    return out
